# revision 21
# baseline (speedup 1.0000x reference)
"""MetaPathGNN kernel for 8 Trainium2 NeuronCores.

Computation (only what the reference output needs — h_b/conv0/edge_ab/x_b are
dead code in the reference):
    msg  = x_a[edge_ba[1]]                      # [E, H] gather
    aggr = segment_sum(msg, edge_ba[0], N)      # [N, H]
    h_a  = relu(aggr @ wl1.T + x_a @ (w01+w11).T + (bl1+b01+b11))
    out  = h_a @ out_w.T + out_b

Sharding: destination nodes are assigned to 8 cores x 98 windows of 64 slots
by degree-balanced bin packing (LPT), so every (core, window) holds ~the same
number of incoming edges and the per-window chunk budgets hit their ceilings
exactly.  Each core gathers the source rows for its own edges from a full
replica of x_a (no collectives), aggregates via one-hot matmuls into PSUM,
then applies the linear layers in feature-major (transposed) layout.

Aggregation scheme per core:
  - 128 edges of one destination window form a "chunk": gathered rows land
    as a [128 edges, 128 feat] SBUF tile (lhsT), a one-hot S [128 edges,
    64 dest] is built on DVE, and matmul(psum_bank[:, win*64:(win+1)*64],
    lhsT=msg, rhs=S) accumulates.  PSUM banks hold 512 destinations
    (8 windows); per bank the first matmul uses start=True (pending-zero
    for the whole bank).
  - x_a rows (256B of f16) are gathered as 64 x f32 via bitcast: the DMA
    cost model charges gathers per *element*, so declaring the row as 64
    f32 elements instead of 128 f16 halves the per-descriptor cost.
    (int64/elem=32 would halve it again in the cost model but produces
    wrong bytes on real hardware.)
  - dma_gather indices are int16; sources are gathered in two passes from
    two bases (rows [0, 32768) and rows [17232, 50000)). Edges whose source
    falls in the overlap [17232, 32768) can go to either pass; per window
    the (passA, passB) chunk budgets are chosen by exhaustive search to
    minimize total chunks under the shared-program constraint.
  - one-hot rows are built 16 chunks at a time with one tensor_tensor
    (is_equal) on DVE.  All three operands keep a packed (stride-1, len-2)
    innermost AP dim — required for DVE 2x vectorization — by storing each
    dest id twice (dest2) and walking iota/dest2 with [(.,nch),(.,W/2),(1,2)]
    access patterns.

The SPMD program is shared by all 8 cores, so per-window chunk budgets are
max'ed across cores; pad slots gather row 0 with dest -1 (all-zero one-hot
row -> no contribution).
"""

import heapq

import numpy as np

P = 8
N = 50000
E = 500000
H = 128
NSH = N // P          # 6250 destination slots per core
W = 64                # destination window width (matmul rhs free dim)
GROUP = 512           # PSUM bank width in fp32 columns
NGROUP = (NSH + GROUP - 1) // GROUP   # 13
NWIN = (NSH + W - 1) // W             # 98 (window 97 partial: 42 slots)
WPG = GROUP // W      # windows per group: 8
BASE_A = 0
LIM_A = 32768         # pass A covers rows [0, 32768)
BASE_B = N - 32768    # 17232; pass B covers rows [17232, 50000)
SB = 16               # chunks per one-hot build op
GATHER_BUFS = 2
S_BUFS = 4
SCRATCH = 98304
CAP_FIRST = 12        # first gather batch (pipeline fill), in chunks
CAPS = (64, 64)       # later batch caps per pass, in chunks
XQ = 4 * GROUP        # xaT prefetch quantum (columns)


def _win_range(g):
    return range(g * WPG, min(g * WPG + WPG, NWIN))


def _bin_rows(dst):
    """Degree-balanced assignment of the 50000 rows to (core, window, slot).

    Longest-processing-time greedy: rows sorted by in-degree descending,
    each placed into the least-loaded non-full (core, window) bin.  Returns
    (perm_core[N], perm_dl[N], rows_of[P, NSH]).
    """
    deg = np.bincount(dst, minlength=N)
    order = np.argsort(-deg, kind="stable")
    nbins = P * NWIN
    cap = np.empty(nbins, np.int64)
    cap.reshape(P, NWIN)[:, :] = W
    cap.reshape(P, NWIN)[:, NWIN - 1] = NSH - (NWIN - 1) * W   # 42
    fill = np.zeros(nbins, np.int64)
    heap = [(0, b) for b in range(nbins)]
    heapq.heapify(heap)
    perm_core = np.empty(N, np.int64)
    perm_dl = np.empty(N, np.int64)
    for row in order:
        while True:
            load, b = heapq.heappop(heap)
            if fill[b] < cap[b]:
                break
        c, w = divmod(b, NWIN)
        perm_core[row] = c
        perm_dl[row] = w * W + fill[b]
        fill[b] += 1
        if fill[b] < cap[b]:
            heapq.heappush(heap, (load + int(deg[row]), b))
    rows_of = np.empty((P, NSH), np.int64)
    rows_of[perm_core, perm_dl] = np.arange(N)
    return perm_core, perm_dl, rows_of


def _pack_edges(dst, src):
    """Bucket edges by (core, window) with an optimized two-pass split.

    Per window, the shared chunk budgets (MA, MB) are chosen by searching MA
    to minimize MA+MB subject to every core fitting its must-A edges in MA
    chunks and the rest in MB chunks (edges in the overlap region are
    assignable to either pass).  Returns (budgets[NWIN,2], group_order, CA,
    CB, per_core, rows_of): per_core[c] holds int16 src index arrays and f16
    duplicated window-local dest arrays for both passes, in processing order
    (groups sorted heaviest-first).
    """
    perm_core, perm_dl, rows_of = _bin_rows(dst)
    core = perm_core[dst]
    dl = perm_dl[dst]
    win = dl // W

    cnt = np.zeros((P, NWIN), np.int64)
    lo = np.zeros((P, NWIN), np.int64)     # must-A counts
    hi = np.zeros((P, NWIN), np.int64)     # must-A + flexible counts
    np.add.at(cnt, (core, win), 1)
    np.add.at(lo, (core[src < BASE_B], win[src < BASE_B]), 1)
    mAok = src < LIM_A
    np.add.at(hi, (core[mAok], win[mAok]), 1)

    budgets = np.zeros((NWIN, 2), np.int64)
    fillA = np.zeros((P, NWIN), np.int64)
    for w in range(NWIN):
        best = None
        ma_lo = int(-(-lo[:, w].max() // 128))
        ma_hi = int(-(-hi[:, w].max() // 128))
        for MA in range(ma_lo, ma_hi + 1):
            capA = np.minimum(128 * MA, hi[:, w])
            MB = max(int((-(-(cnt[:, w] - capA) // 128)).max()), 0)
            if best is None or MA + MB < best[0] + best[1]:
                best = (MA, MB)
        MA, MB = best
        budgets[w] = (MA, MB)
        capA = np.minimum(128 * MA, hi[:, w])
        need = cnt[:, w] - 128 * MB
        fillA[:, w] = np.maximum(capA, need)
        assert (fillA[:, w] >= lo[:, w]).all() and (fillA[:, w] <= capA).all()

    # process heaviest groups first so the post-gather tail is minimal
    gtot = np.array([budgets[list(_win_range(g))].sum() for g in range(NGROUP)])
    group_order = list(np.argsort(-gtot, kind="stable"))

    CA = int(budgets[:, 0].sum())
    CB = int(budgets[:, 1].sum())

    # window sequence in processing order -> stream slot offsets per pass
    wseq = [w for g in group_order for w in _win_range(g)]
    offs = np.zeros((NWIN, 2), np.int64)
    for p in range(2):
        acc = 0
        for w in wseq:
            offs[w, p] = acc
            acc += int(budgets[w, p]) * 128

    per_core = []
    for c in range(P):
        m = core == c
        dlc, winc, sc = dl[m], win[m], src[m]
        # order edges by (window, src-class); must-A first, then flexible,
        # then must-B, so a rank threshold splits the passes
        cls = np.full(len(sc), 1, np.int64)
        cls[sc < BASE_B] = 0
        cls[sc >= LIM_A] = 2
        order = np.lexsort((cls, winc))
        dlc, winc, sc = dlc[order], winc[order], sc[order]

        first = np.zeros(NWIN, np.int64)
        bc = np.bincount(winc, minlength=NWIN)
        first[1:] = np.cumsum(bc)[:-1]
        rank = np.arange(len(winc)) - first[winc]
        fa = fillA[c][winc]
        isB = rank >= fa
        slot = np.where(isB, offs[winc, 1] + (rank - fa), offs[winc, 0] + rank)

        arrs = {}
        for p, name, base, CP in ((0, "A", BASE_A, CA), (1, "B", BASE_B, CB)):
            L = CP * 128
            idx = np.zeros(L, np.int64)          # pad -> row 0 of the base
            dest = np.full(L, -1.0, np.float16)  # pad -> no one-hot match
            mm = isB == (p == 1)
            idx[slot[mm]] = sc[mm] - base
            dest[slot[mm]] = (dlc[mm] - winc[mm] * W).astype(np.float16)
            assert idx.min() >= 0 and idx.max() < 32768
            arrs["idx" + name] = idx.astype(np.int16)
            # [slot, chunk] layout with each value duplicated along chunks
            d2 = np.repeat(dest.reshape(-1, 128).T, 2, axis=1)
            arrs["dest" + name] = np.ascontiguousarray(d2)
        per_core.append(arrs)

    return budgets, group_order, CA, CB, per_core, rows_of


def _wrap_idx(idx):
    """dma_gather index layout: element i at [i % 16, i // 16], tiled to 128
    partitions."""
    w = np.ascontiguousarray(idx.reshape(-1, 16).T)  # [16, L/16]
    return np.tile(w, (8, 1))


def _batches(budgets, group_order, p):
    """Cut a pass stream into gather batches at group boundaries.

    The last two groups get their own small batches so the post-gather
    consumption tail is short."""
    per_group = [int(budgets[list(_win_range(g)), p].sum()) for g in group_order]
    blist, start, cur = [], 0, 0
    caps = [CAP_FIRST, 32]
    ng = len(per_group)
    for i, n in enumerate(per_group):
        solo = i >= ng - 2
        cap = caps[len(blist)] if len(blist) < len(caps) else CAPS[p]
        if cur and (solo or cur + n > cap):
            blist.append((start, cur))
            start, cur = start + cur, 0
        cur += n
        if solo and cur:
            blist.append((start, cur))
            start, cur = start + cur, 0
    if cur:
        blist.append((start, cur))
    return blist


def _build_program(budgets, group_order, CA, CB):
    import concourse.bacc as bacc
    import concourse.tile as tile
    import concourse.mybir as mybir
    from concourse.ap import AP

    F32 = mybir.dt.float32
    F16 = mybir.dt.float16
    I16 = mybir.dt.int16
    NCOL = NGROUP * GROUP

    nc = bacc.Bacc("TRN2", num_swdge_queues=4, dynamic_dma_scratch_size=SCRATCH)
    xa32_d = nc.dram_tensor("xa32", [N, 64], F32, kind="ExternalInput")
    idxA_d = nc.dram_tensor("idxA", [128, CA * 8], I16, kind="ExternalInput")
    idxB_d = nc.dram_tensor("idxB", [128, CB * 8], I16, kind="ExternalInput")
    destA_d = nc.dram_tensor("destA", [128, 2 * CA], F16, kind="ExternalInput")
    destB_d = nc.dram_tensor("destB", [128, 2 * CB], F16, kind="ExternalInput")
    wagg_d = nc.dram_tensor("wagg", [H, H], F16, kind="ExternalInput")
    wx_d = nc.dram_tensor("wx", [H, H], F16, kind="ExternalInput")
    wo_d = nc.dram_tensor("wo", [H, H], F16, kind="ExternalInput")
    bh_d = nc.dram_tensor("bh", [H, 1], F32, kind="ExternalInput")
    bo_d = nc.dram_tensor("bo", [1, H], F16, kind="ExternalInput")
    boc_d = nc.dram_tensor("boc", [H, 1], F32, kind="ExternalInput")
    ones_d = nc.dram_tensor("ones", [1, GROUP], F16, kind="ExternalInput")
    iota_d = nc.dram_tensor("iota", [128, W], F16, kind="ExternalInput")
    # xaT columns are laid out in PROCESSING order (group_order), so the
    # prefetch quanta are contiguous slices.
    xaT_d = nc.dram_tensor("xaT", [H, NCOL], F16, kind="ExternalInput")
    outT_d = nc.dram_tensor("outT", [H, NSH], F16, kind="ExternalOutput")

    CN = [CA, CB]
    batches = [_batches(budgets, group_order, p) for p in range(2)]
    idx_d = [idxA_d, idxB_d]
    dest_d = [destA_d, destB_d]
    base = [(BASE_A, BASE_A + 32768), (BASE_B, N)]

    with tile.TileContext(nc) as tc:
        with (
            tc.tile_pool(name="const", bufs=1) as constp,
            tc.tile_pool(name="gath", bufs=GATHER_BUFS) as gathp,
            tc.tile_pool(name="xq", bufs=2) as xqp,
            tc.tile_pool(name="sbld", bufs=S_BUFS) as spool,
            tc.tile_pool(name="post", bufs=2) as postp,
            tc.tile_pool(name="ps", bufs=2, space="PSUM") as psump,
        ):
            # first-batch indices lead so the first gather dispatches ASAP
            dest_t = []
            idx_t = []
            for p in range(2):
                dt_ = constp.tile([128, 2 * CN[p]], F16, tag=f"dest{p}",
                                  name=f"dest{p}")
                it_ = constp.tile([128, CN[p] * 8], I16, tag=f"idx{p}",
                                  name=f"idx{p}")
                dest_t.append(dt_)
                idx_t.append(it_)
            cuts = [batches[p][0][1] * 8 for p in range(2)]
            nc.scalar.dma_start(idx_t[0][:, :cuts[0]], idx_d[0][:, :cuts[0]])
            iota_t = constp.tile([128, W], F16)
            nc.scalar.dma_start(idx_t[1][:, :cuts[1]], idx_d[1][:, :cuts[1]])
            nc.sync.dma_start(iota_t[:], iota_d[:])
            nc.sync.dma_start(dest_t[0][:], dest_d[0][:])
            nc.sync.dma_start(dest_t[1][:], dest_d[1][:])
            for p in range(2):
                nc.sync.dma_start(idx_t[p][:, cuts[p]:], idx_d[p][:, cuts[p]:])
            wagg_t = constp.tile([H, H], F16, tag="wagg")
            wx_t = constp.tile([H, H], F16, tag="wx")
            wo_t = constp.tile([H, H], F16, tag="wo")
            bh_t = constp.tile([H, 1], F32, tag="bh")
            bo_t = constp.tile([1, H], F16, tag="bo")
            boc_t = constp.tile([H, 1], F32, tag="boc")
            ones_t = constp.tile([1, GROUP], F16, tag="ones")
            for t, dd in ((wagg_t, wagg_d), (wx_t, wx_d), (wo_t, wo_d),
                          (bh_t, bh_d), (bo_t, bo_d), (boc_t, boc_d),
                          (ones_t, ones_d)):
                nc.sync.dma_start(t[:], dd[:])

            # streaming state per pass: current batch tile / S tile
            cur_batch = [None, None]
            cur_s = [None, None]
            batch_pos = [0, 0]
            batch_start = [0, 0]

            def chunk_tiles(p, c):
                """(lhsT msg AP, rhs S AP) for chunk c of pass p; emits the
                gather / S-build on first touch of their batch / S tile."""
                if cur_batch[p] is None or c >= batch_start[p] + cur_batch[p].shape[1]:
                    start, nch = batches[p][batch_pos[p]]
                    assert start == c, (p, c, start)
                    batch_pos[p] += 1
                    batch_start[p] = start
                    t = gathp.tile([128, CAPS[p], H], F16, tag=f"g{p}")
                    t = t[:, :nch, :]
                    lo, hi = base[p]
                    nc.gpsimd.dma_gather(
                        t[:].bitcast(mybir.dt.float32),
                        xa32_d[lo:hi, :],
                        idx_t[p][:, start * 8:(start + nch) * 8],
                        nch * 128,
                        nch * 128,
                        64,
                        single_packet=False,
                        queue_num=0,
                    )
                    cur_batch[p] = t
                r = c - batch_start[p]
                sb, sr = divmod(c, SB)
                if sr == 0:
                    nsb = min(SB, CN[p] - c)
                    st = spool.tile([128, SB, W], F16, tag=f"s{p}")
                    # packed-innermost-dim APs: iota walks [(0,nsb),(2,W/2),
                    # (1,2)], dest2 walks [(2,nsb),(0,W/2),(1,2)] — every
                    # operand keeps a (stride 1, len 2) last dim so DVE
                    # vectorization applies.
                    i0 = iota_t[:]
                    in0 = AP(i0.tensor, i0.offset,
                             [i0.ap[0], (0, nsb), (2, W // 2), (1, 2)])
                    d0 = dest_t[p][:, 2 * c: 2 * (c + nsb)]
                    in1 = AP(d0.tensor, d0.offset,
                             [d0.ap[0], (2, nsb), (0, W // 2), (1, 2)])
                    nc.vector.tensor_tensor(
                        out=st[:, :nsb, :], in0=in0, in1=in1,
                        op=mybir.AluOpType.is_equal,
                    )
                    cur_s[p] = st
                return cur_batch[p][:, r, :], cur_s[p][:, sr, :]

            relu = mybir.ActivationFunctionType.Relu
            copyf = mybir.ActivationFunctionType.Copy
            consumed = [0, 0]
            xq_tile = [None]

            for gi, g in enumerate(group_order):
                glo = g * GROUP
                ncols = min(GROUP, NSH - glo)     # 106 for the last group
                if gi % (XQ // GROUP) == 0:
                    qlo = gi * GROUP
                    qn = min(XQ, NCOL - qlo)
                    xt = xqp.tile([128, XQ], F16, tag="xq", name=f"xq{gi}")
                    nc.sync.dma_start(xt[:, :qn], xaT_d[:, qlo:qlo + qn])
                    xq_tile[0] = xt
                xaT_g = xq_tile[0][:, (gi % (XQ // GROUP)) * GROUP:
                                   (gi % (XQ // GROUP)) * GROUP + GROUP]
                nmm = sum(int(budgets[w, p]) for w in _win_range(g) for p in range(2))
                aggr_ps = psump.tile([128, GROUP], F32, tag="aggr", bufs=4)
                mmi = 0
                for w in _win_range(g):
                    w8 = w - g * WPG
                    for p in range(2):
                        for _ in range(int(budgets[w, p])):
                            lhsT, rhs = chunk_tiles(p, consumed[p])
                            consumed[p] += 1
                            nc.tensor.matmul(
                                aggr_ps[:, w8 * W:(w8 + 1) * W], lhsT, rhs,
                                start=(mmi == 0), stop=(mmi == nmm - 1),
                            )
                            mmi += 1
                mw = ((ncols + W - 1) // W) * W   # post-stage width
                z_ps = psump.tile([128, GROUP], F32, tag="z")
                if nmm:
                    aggr_sb = postp.tile([128, GROUP], F16, tag="aggr_sb")
                    nc.scalar.activation(aggr_sb[:, :mw], aggr_ps[:, :mw], copyf)
                    nc.tensor.matmul(z_ps[:, :mw], wagg_t[:], aggr_sb[:, :mw],
                                     start=True, stop=False)
                    nc.tensor.matmul(z_ps[:, :mw], wx_t[:], xaT_g[:, :mw],
                                     start=False, stop=True)
                else:
                    nc.tensor.matmul(z_ps[:, :mw], wx_t[:], xaT_g[:, :mw],
                                     start=True, stop=True)
                h_sb = postp.tile([128, GROUP], F16, tag="h")
                nc.scalar.activation(h_sb[:, :mw], z_ps[:, :mw], relu,
                                     bias=bh_t[:, 0:1])
                o_ps = psump.tile([128, GROUP], F32, tag="o")
                o_sb = postp.tile([128, GROUP], F16, tag="osb")
                if gi % 2 == 0:
                    nc.tensor.matmul(o_ps[:, :mw], wo_t[:], h_sb[:, :mw],
                                     start=True, stop=True)
                    nc.vector.tensor_scalar(
                        out=o_sb[:, :mw], in0=o_ps[:, :mw],
                        scalar1=boc_t[:, 0:1], scalar2=None,
                        op0=mybir.AluOpType.add,
                    )
                else:
                    nc.tensor.matmul(o_ps[:, :mw], wo_t[:], h_sb[:, :mw],
                                     start=True, stop=False)
                    nc.tensor.matmul(o_ps[:, :mw], bo_t[:], ones_t[:, :mw],
                                     start=False, stop=True)
                    nc.scalar.activation(o_sb[:, :mw], o_ps[:, :mw], copyf)
                nc.sync.dma_start(outT_d[:, glo:glo + ncols], o_sb[:, :ncols])

    nc.compile()
    return nc


def prepare(inputs):
    """Host-side packing: returns (nc, in_maps)."""
    x_a = np.ascontiguousarray(np.asarray(inputs["x_a"], dtype=np.float32))
    eb = np.asarray(inputs["edge_ba"])
    dst = eb[0].astype(np.int64)
    src = eb[1].astype(np.int64)

    wagg = np.ascontiguousarray(np.asarray(inputs["conv1_wl_w"], np.float32).T.astype(np.float16))
    wx = np.ascontiguousarray(
        (np.asarray(inputs["conv1_w0_w"], np.float32)
         + np.asarray(inputs["conv1_w1_w"], np.float32)).T.astype(np.float16))
    bh = (np.asarray(inputs["conv1_wl_b"], np.float32)
          + np.asarray(inputs["conv1_w0_b"], np.float32)
          + np.asarray(inputs["conv1_w1_b"], np.float32)).reshape(H, 1)
    wo = np.ascontiguousarray(np.asarray(inputs["out_w"], np.float32).T.astype(np.float16))
    bo = np.asarray(inputs["out_b"], np.float32).reshape(1, H).astype(np.float16)
    boc = np.asarray(inputs["out_b"], np.float32).reshape(H, 1)
    iota = np.ascontiguousarray(
        np.tile(np.arange(W, dtype=np.float16)[None, :], (128, 1)))
    xa16 = np.ascontiguousarray(x_a.astype(np.float16))
    xa32 = xa16.view(np.float32)   # [N, 64] bit view

    budgets, group_order, CA, CB, per_core, rows_of = _pack_edges(dst, src)
    nc = _build_program(budgets, group_order, CA, CB)

    NCOL = NGROUP * GROUP
    in_maps = []
    for c in range(P):
        xn = np.zeros((H, NCOL), np.float16)
        xn[:, :NSH] = xa16[rows_of[c]].T
        # processing-order column layout for contiguous prefetch quanta
        xaT = np.empty((H, NCOL), np.float16)
        for gi, g in enumerate(group_order):
            xaT[:, gi * GROUP:(gi + 1) * GROUP] = xn[:, g * GROUP:(g + 1) * GROUP]
        a = per_core[c]
        in_maps.append({
            "xa32": xa32,
            "xaT": xaT,
            "idxA": _wrap_idx(a["idxA"]),
            "idxB": _wrap_idx(a["idxB"]),
            "destA": a["destA"],
            "destB": a["destB"],
            "wagg": wagg, "wx": wx, "wo": wo, "bh": bh, "bo": bo,
            "boc": boc,
            "ones": np.ones((1, GROUP), np.float16),
            "iota": iota,
        })
    return nc, in_maps, rows_of


def assemble(results, rows_of):
    out = np.empty((N, H), np.float32)
    for c in range(P):
        out[rows_of[c]] = results[c]["outT"].T.astype(np.float32)
    return out


def kernel(**inputs):
    from concourse.bass_utils import run_bass_kernel_spmd

    nc, in_maps, rows_of = prepare(inputs)
    r = run_bass_kernel_spmd(nc, in_maps, list(range(P)))
    return assemble(r.results, rows_of)


# revision 22
# speedup vs baseline: 1.0342x; 1.0342x over previous
"""MetaPathGNN kernel for 8 Trainium2 NeuronCores.

Computation (only what the reference output needs — h_b/conv0/edge_ab/x_b are
dead code in the reference):
    msg  = x_a[edge_ba[1]]                      # [E, H] gather
    aggr = segment_sum(msg, edge_ba[0], N)      # [N, H]
    h_a  = relu(aggr @ wl1.T + x_a @ (w01+w11).T + (bl1+b01+b11))
    out  = h_a @ out_w.T + out_b

Sharding: destination nodes are assigned to 8 cores x 98 windows of 64 slots
by degree-balanced bin packing (LPT), so every (core, window) holds ~the same
number of incoming edges and the per-window chunk budgets hit their ceilings
exactly.  Each core gathers the source rows for its own edges from a full
replica of x_a (no collectives), aggregates via one-hot matmuls into PSUM,
then applies the linear layers in feature-major (transposed) layout.

Aggregation scheme per core:
  - 128 edges of one destination window form a "chunk": gathered rows land
    as a [128 edges, 128 feat] SBUF tile (lhsT), a one-hot S [128 edges,
    64 dest] is built on DVE, and matmul(psum_bank[:, win*64:(win+1)*64],
    lhsT=msg, rhs=S) accumulates.  PSUM banks hold 512 destinations
    (8 windows); per bank the first matmul uses start=True (pending-zero
    for the whole bank).
  - x_a rows (256B of f16) are gathered as 64 x f32 via bitcast: the DMA
    cost model charges gathers per *element*, so declaring the row as 64
    f32 elements instead of 128 f16 halves the per-descriptor cost.
    (int64/elem=32 would halve it again in the cost model but produces
    wrong bytes on real hardware.)
  - dma_gather indices are int16; sources are gathered in two passes from
    two bases (rows [0, 32768) and rows [17232, 50000)). Edges whose source
    falls in the overlap [17232, 32768) can go to either pass; per window
    the (passA, passB) chunk budgets are chosen by exhaustive search to
    minimize total chunks under the shared-program constraint.
  - one-hot rows are built 16 chunks at a time with one tensor_tensor
    (is_equal) on DVE.  All three operands keep a packed (stride-1, len-2)
    innermost AP dim — required for DVE 2x vectorization — by storing each
    dest id twice (dest2) and walking iota/dest2 with [(.,nch),(.,W/2),(1,2)]
    access patterns.

The SPMD program is shared by all 8 cores, so per-window chunk budgets are
max'ed across cores; pad slots gather row 0 with dest -1 (all-zero one-hot
row -> no contribution).
"""

import heapq

import numpy as np

P = 8
N = 50000
E = 500000
H = 128
NSH = N // P          # 6250 destination slots per core
W = 64                # destination window width (matmul rhs free dim)
GROUP = 512           # PSUM bank width in fp32 columns
NGROUP = (NSH + GROUP - 1) // GROUP   # 13
NWIN = (NSH + W - 1) // W             # 98 (window 97 partial: 42 slots)
WPG = GROUP // W      # windows per group: 8
BASE_A = 0
LIM_A = 32768         # pass A covers rows [0, 32768)
BASE_B = N - 32768    # 17232; pass B covers rows [17232, 50000)
SB = 16               # chunks per one-hot build op
GATHER_BUFS = 2
S_BUFS = 4
SCRATCH = 98304
CAP_FIRST = 12        # first gather batch (pipeline fill), in chunks
CAPS = (64, 64)       # later batch caps per pass, in chunks
XQ = 4 * GROUP        # xaT prefetch quantum (columns)


def _win_range(g):
    return range(g * WPG, min(g * WPG + WPG, NWIN))


def _bin_rows(dst):
    """Degree-balanced assignment of the 50000 rows to (core, window, slot).

    Longest-processing-time greedy: rows sorted by in-degree descending,
    each placed into the least-loaded non-full (core, window) bin.  Returns
    (perm_core[N], perm_dl[N], rows_of[P, NSH]).
    """
    deg = np.bincount(dst, minlength=N)
    order = np.argsort(-deg, kind="stable")
    nbins = P * NWIN
    cap = np.empty(nbins, np.int64)
    cap.reshape(P, NWIN)[:, :] = W
    cap.reshape(P, NWIN)[:, NWIN - 1] = NSH - (NWIN - 1) * W   # 42
    fill = np.zeros(nbins, np.int64)
    heap = [(0, b) for b in range(nbins)]
    heapq.heapify(heap)
    perm_core = np.empty(N, np.int64)
    perm_dl = np.empty(N, np.int64)
    for row in order:
        while True:
            load, b = heapq.heappop(heap)
            if fill[b] < cap[b]:
                break
        c, w = divmod(b, NWIN)
        perm_core[row] = c
        perm_dl[row] = w * W + fill[b]
        fill[b] += 1
        if fill[b] < cap[b]:
            heapq.heappush(heap, (load + int(deg[row]), b))
    rows_of = np.empty((P, NSH), np.int64)
    rows_of[perm_core, perm_dl] = np.arange(N)
    return perm_core, perm_dl, rows_of


def _pack_edges(dst, src):
    """Bucket edges by (core, window) with an optimized two-pass split.

    Per window, the shared chunk budgets (MA, MB) are chosen by searching MA
    to minimize MA+MB subject to every core fitting its must-A edges in MA
    chunks and the rest in MB chunks (edges in the overlap region are
    assignable to either pass).  Returns (budgets[NWIN,2], group_order, CA,
    CB, per_core, rows_of): per_core[c] holds int16 src index arrays and f16
    duplicated window-local dest arrays for both passes, in processing order
    (groups sorted heaviest-first).
    """
    perm_core, perm_dl, rows_of = _bin_rows(dst)
    core = perm_core[dst]
    dl = perm_dl[dst]
    win = dl // W

    cnt = np.zeros((P, NWIN), np.int64)
    lo = np.zeros((P, NWIN), np.int64)     # must-A counts
    hi = np.zeros((P, NWIN), np.int64)     # must-A + flexible counts
    np.add.at(cnt, (core, win), 1)
    np.add.at(lo, (core[src < BASE_B], win[src < BASE_B]), 1)
    mAok = src < LIM_A
    np.add.at(hi, (core[mAok], win[mAok]), 1)

    budgets = np.zeros((NWIN, 2), np.int64)
    fillA = np.zeros((P, NWIN), np.int64)
    for w in range(NWIN):
        best = None
        ma_lo = int(-(-lo[:, w].max() // 128))
        ma_hi = int(-(-hi[:, w].max() // 128))
        for MA in range(ma_lo, ma_hi + 1):
            capA = np.minimum(128 * MA, hi[:, w])
            MB = max(int((-(-(cnt[:, w] - capA) // 128)).max()), 0)
            if best is None or MA + MB < best[0] + best[1]:
                best = (MA, MB)
        MA, MB = best
        budgets[w] = (MA, MB)
        capA = np.minimum(128 * MA, hi[:, w])
        need = cnt[:, w] - 128 * MB
        fillA[:, w] = np.maximum(capA, need)
        assert (fillA[:, w] >= lo[:, w]).all() and (fillA[:, w] <= capA).all()

    # process heaviest groups first so the post-gather tail is minimal
    gtot = np.array([budgets[list(_win_range(g))].sum() for g in range(NGROUP)])
    group_order = list(np.argsort(-gtot, kind="stable"))

    CA = int(budgets[:, 0].sum())
    CB = int(budgets[:, 1].sum())

    # window sequence in processing order -> stream slot offsets per pass
    wseq = [w for g in group_order for w in _win_range(g)]
    offs = np.zeros((NWIN, 2), np.int64)
    for p in range(2):
        acc = 0
        for w in wseq:
            offs[w, p] = acc
            acc += int(budgets[w, p]) * 128

    per_core = []
    for c in range(P):
        m = core == c
        dlc, winc, sc = dl[m], win[m], src[m]
        # order edges by (window, src-class); must-A first, then flexible,
        # then must-B, so a rank threshold splits the passes
        cls = np.full(len(sc), 1, np.int64)
        cls[sc < BASE_B] = 0
        cls[sc >= LIM_A] = 2
        order = np.lexsort((cls, winc))
        dlc, winc, sc = dlc[order], winc[order], sc[order]

        first = np.zeros(NWIN, np.int64)
        bc = np.bincount(winc, minlength=NWIN)
        first[1:] = np.cumsum(bc)[:-1]
        rank = np.arange(len(winc)) - first[winc]
        fa = fillA[c][winc]
        isB = rank >= fa
        slot = np.where(isB, offs[winc, 1] + (rank - fa), offs[winc, 0] + rank)

        arrs = {}
        for p, name, base, CP in ((0, "A", BASE_A, CA), (1, "B", BASE_B, CB)):
            L = CP * 128
            idx = np.zeros(L, np.int64)          # pad -> row 0 of the base
            dest = np.full(L, -1.0, np.float16)  # pad -> no one-hot match
            mm = isB == (p == 1)
            idx[slot[mm]] = sc[mm] - base
            dest[slot[mm]] = (dlc[mm] - winc[mm] * W).astype(np.float16)
            assert idx.min() >= 0 and idx.max() < 32768
            arrs["idx" + name] = idx.astype(np.int16)
            # [slot, chunk] layout with each value duplicated along chunks
            d2 = np.repeat(dest.reshape(-1, 128).T, 2, axis=1)
            arrs["dest" + name] = np.ascontiguousarray(d2)
        per_core.append(arrs)

    return budgets, group_order, CA, CB, per_core, rows_of


def _wrap_idx(idx):
    """dma_gather index layout: element i at [i % 16, i // 16], tiled to 128
    partitions."""
    w = np.ascontiguousarray(idx.reshape(-1, 16).T)  # [16, L/16]
    return np.tile(w, (8, 1))


def _batches(budgets, group_order, p):
    """Cut a pass stream into gather batches at group boundaries.

    The last two groups get their own small batches so the post-gather
    consumption tail is short."""
    per_group = [int(budgets[list(_win_range(g)), p].sum()) for g in group_order]
    blist, start, cur = [], 0, 0
    caps = [CAP_FIRST, 32]
    ng = len(per_group)
    for i, n in enumerate(per_group):
        solo = i >= ng - 2
        cap = caps[len(blist)] if len(blist) < len(caps) else CAPS[p]
        if cur and (solo or cur + n > cap):
            blist.append((start, cur))
            start, cur = start + cur, 0
        cur += n
        if solo and cur:
            blist.append((start, cur))
            start, cur = start + cur, 0
    if cur:
        blist.append((start, cur))
    return blist


def _build_program(budgets, group_order, CA, CB):
    import concourse.bacc as bacc
    import concourse.tile as tile
    import concourse.mybir as mybir
    from concourse.ap import AP

    F32 = mybir.dt.float32
    F16 = mybir.dt.float16
    I16 = mybir.dt.int16
    NCOL = NGROUP * GROUP

    nc = bacc.Bacc("TRN2", num_swdge_queues=4, dynamic_dma_scratch_size=SCRATCH)
    xa32_d = nc.dram_tensor("xa32", [N, 64], F32, kind="ExternalInput")
    idxA_d = nc.dram_tensor("idxA", [128, CA * 8], I16, kind="ExternalInput")
    idxB_d = nc.dram_tensor("idxB", [128, CB * 8], I16, kind="ExternalInput")
    destA_d = nc.dram_tensor("destA", [128, 2 * CA], F16, kind="ExternalInput")
    destB_d = nc.dram_tensor("destB", [128, 2 * CB], F16, kind="ExternalInput")
    wagg_d = nc.dram_tensor("wagg", [H, H], F16, kind="ExternalInput")
    wx_d = nc.dram_tensor("wx", [H, H], F16, kind="ExternalInput")
    wo_d = nc.dram_tensor("wo", [H, H], F16, kind="ExternalInput")
    bh_d = nc.dram_tensor("bh", [H, 1], F32, kind="ExternalInput")
    bo_d = nc.dram_tensor("bo", [1, H], F16, kind="ExternalInput")
    boc_d = nc.dram_tensor("boc", [H, 1], F32, kind="ExternalInput")
    ones_d = nc.dram_tensor("ones", [1, GROUP], F16, kind="ExternalInput")
    iota_d = nc.dram_tensor("iota", [128, W], F16, kind="ExternalInput")
    # xaT columns are laid out in PROCESSING order (group_order), so the
    # prefetch quanta are contiguous slices.
    xaT_d = nc.dram_tensor("xaT", [H, NCOL], F16, kind="ExternalInput")
    outT_d = nc.dram_tensor("outT", [H, NSH], F16, kind="ExternalOutput")

    CN = [CA, CB]
    batches = [_batches(budgets, group_order, p) for p in range(2)]
    idx_d = [idxA_d, idxB_d]
    dest_d = [destA_d, destB_d]
    base = [(BASE_A, BASE_A + 32768), (BASE_B, N)]

    with tile.TileContext(nc) as tc:
        with (
            tc.tile_pool(name="const", bufs=1) as constp,
            tc.tile_pool(name="gath", bufs=GATHER_BUFS) as gathp,
            tc.tile_pool(name="xq", bufs=2) as xqp,
            tc.tile_pool(name="sbld", bufs=S_BUFS) as spool,
            tc.tile_pool(name="post", bufs=2) as postp,
            tc.tile_pool(name="ps", bufs=2, space="PSUM") as psump,
        ):
            # first-batch indices lead so the first gather dispatches ASAP
            dest_t = []
            idx_t = []
            for p in range(2):
                dt_ = constp.tile([128, 2 * CN[p]], F16, tag=f"dest{p}",
                                  name=f"dest{p}")
                it_ = constp.tile([128, CN[p] * 8], I16, tag=f"idx{p}",
                                  name=f"idx{p}")
                dest_t.append(dt_)
                idx_t.append(it_)
            cuts = [batches[p][0][1] * 8 for p in range(2)]
            nc.sync.dma_start(idx_t[0][:, :cuts[0]], idx_d[0][:, :cuts[0]])
            iota_t = constp.tile([128, W], F16)
            nc.sync.dma_start(iota_t[:], iota_d[:])
            nc.sync.dma_start(dest_t[0][:], dest_d[0][:])
            nc.sync.dma_start(idx_t[1][:, :cuts[1]], idx_d[1][:, :cuts[1]])
            nc.sync.dma_start(dest_t[1][:], dest_d[1][:])
            for p in range(2):
                nc.sync.dma_start(idx_t[p][:, cuts[p]:], idx_d[p][:, cuts[p]:])
            wagg_t = constp.tile([H, H], F16, tag="wagg")
            wx_t = constp.tile([H, H], F16, tag="wx")
            wo_t = constp.tile([H, H], F16, tag="wo")
            bh_t = constp.tile([H, 1], F32, tag="bh")
            bo_t = constp.tile([1, H], F16, tag="bo")
            boc_t = constp.tile([H, 1], F32, tag="boc")
            ones_t = constp.tile([1, GROUP], F16, tag="ones")
            for t, dd in ((wagg_t, wagg_d), (wx_t, wx_d), (wo_t, wo_d),
                          (bh_t, bh_d), (bo_t, bo_d), (boc_t, boc_d),
                          (ones_t, ones_d)):
                nc.sync.dma_start(t[:], dd[:])

            # streaming state per pass: current batch tile / S tile
            cur_batch = [None, None]
            cur_s = [None, None]
            batch_pos = [0, 0]
            batch_start = [0, 0]

            def chunk_tiles(p, c):
                """(lhsT msg AP, rhs S AP) for chunk c of pass p; emits the
                gather / S-build on first touch of their batch / S tile."""
                if cur_batch[p] is None or c >= batch_start[p] + cur_batch[p].shape[1]:
                    start, nch = batches[p][batch_pos[p]]
                    assert start == c, (p, c, start)
                    batch_pos[p] += 1
                    batch_start[p] = start
                    t = gathp.tile([128, CAPS[p], H], F16, tag=f"g{p}")
                    t = t[:, :nch, :]
                    lo, hi = base[p]
                    nc.gpsimd.dma_gather(
                        t[:].bitcast(mybir.dt.float32),
                        xa32_d[lo:hi, :],
                        idx_t[p][:, start * 8:(start + nch) * 8],
                        nch * 128,
                        nch * 128,
                        64,
                        single_packet=False,
                        queue_num=0,
                    )
                    cur_batch[p] = t
                r = c - batch_start[p]
                sb, sr = divmod(c, SB)
                if sr == 0:
                    nsb = min(SB, CN[p] - c)
                    st = spool.tile([128, SB, W], F16, tag=f"s{p}")
                    # packed-innermost-dim APs: iota walks [(0,nsb),(2,W/2),
                    # (1,2)], dest2 walks [(2,nsb),(0,W/2),(1,2)] — every
                    # operand keeps a (stride 1, len 2) last dim so DVE
                    # vectorization applies.
                    i0 = iota_t[:]
                    in0 = AP(i0.tensor, i0.offset,
                             [i0.ap[0], (0, nsb), (2, W // 2), (1, 2)])
                    d0 = dest_t[p][:, 2 * c: 2 * (c + nsb)]
                    in1 = AP(d0.tensor, d0.offset,
                             [d0.ap[0], (2, nsb), (0, W // 2), (1, 2)])
                    nc.vector.tensor_tensor(
                        out=st[:, :nsb, :], in0=in0, in1=in1,
                        op=mybir.AluOpType.is_equal,
                    )
                    cur_s[p] = st
                return cur_batch[p][:, r, :], cur_s[p][:, sr, :]

            relu = mybir.ActivationFunctionType.Relu
            copyf = mybir.ActivationFunctionType.Copy
            consumed = [0, 0]
            xq_tile = [None]

            for gi, g in enumerate(group_order):
                glo = g * GROUP
                ncols = min(GROUP, NSH - glo)     # 106 for the last group
                if gi % (XQ // GROUP) == 0:
                    qlo = gi * GROUP
                    qn = min(XQ, NCOL - qlo)
                    xt = xqp.tile([128, XQ], F16, tag="xq", name=f"xq{gi}")
                    nc.sync.dma_start(xt[:, :qn], xaT_d[:, qlo:qlo + qn])
                    xq_tile[0] = xt
                xaT_g = xq_tile[0][:, (gi % (XQ // GROUP)) * GROUP:
                                   (gi % (XQ // GROUP)) * GROUP + GROUP]
                nmm = sum(int(budgets[w, p]) for w in _win_range(g) for p in range(2))
                aggr_ps = psump.tile([128, GROUP], F32, tag="aggr", bufs=4)
                mmi = 0
                for w in _win_range(g):
                    w8 = w - g * WPG
                    for p in range(2):
                        for _ in range(int(budgets[w, p])):
                            lhsT, rhs = chunk_tiles(p, consumed[p])
                            consumed[p] += 1
                            nc.tensor.matmul(
                                aggr_ps[:, w8 * W:(w8 + 1) * W], lhsT, rhs,
                                start=(mmi == 0), stop=(mmi == nmm - 1),
                            )
                            mmi += 1
                mw = ((ncols + W - 1) // W) * W   # post-stage width
                z_ps = psump.tile([128, GROUP], F32, tag="z")
                if nmm:
                    aggr_sb = postp.tile([128, GROUP], F16, tag="aggr_sb")
                    nc.scalar.activation(aggr_sb[:, :mw], aggr_ps[:, :mw], copyf)
                    nc.tensor.matmul(z_ps[:, :mw], wagg_t[:], aggr_sb[:, :mw],
                                     start=True, stop=False)
                    nc.tensor.matmul(z_ps[:, :mw], wx_t[:], xaT_g[:, :mw],
                                     start=False, stop=True)
                else:
                    nc.tensor.matmul(z_ps[:, :mw], wx_t[:], xaT_g[:, :mw],
                                     start=True, stop=True)
                h_sb = postp.tile([128, GROUP], F16, tag="h")
                nc.scalar.activation(h_sb[:, :mw], z_ps[:, :mw], relu,
                                     bias=bh_t[:, 0:1])
                o_ps = psump.tile([128, GROUP], F32, tag="o")
                o_sb = postp.tile([128, GROUP], F16, tag="osb")
                if gi % 2 == 0:
                    nc.tensor.matmul(o_ps[:, :mw], wo_t[:], h_sb[:, :mw],
                                     start=True, stop=True)
                    nc.vector.tensor_scalar(
                        out=o_sb[:, :mw], in0=o_ps[:, :mw],
                        scalar1=boc_t[:, 0:1], scalar2=None,
                        op0=mybir.AluOpType.add,
                    )
                else:
                    nc.tensor.matmul(o_ps[:, :mw], wo_t[:], h_sb[:, :mw],
                                     start=True, stop=False)
                    nc.tensor.matmul(o_ps[:, :mw], bo_t[:], ones_t[:, :mw],
                                     start=False, stop=True)
                    nc.scalar.activation(o_sb[:, :mw], o_ps[:, :mw], copyf)
                nc.sync.dma_start(outT_d[:, glo:glo + ncols], o_sb[:, :ncols])

    nc.compile()
    return nc


def prepare(inputs):
    """Host-side packing: returns (nc, in_maps)."""
    x_a = np.ascontiguousarray(np.asarray(inputs["x_a"], dtype=np.float32))
    eb = np.asarray(inputs["edge_ba"])
    dst = eb[0].astype(np.int64)
    src = eb[1].astype(np.int64)

    wagg = np.ascontiguousarray(np.asarray(inputs["conv1_wl_w"], np.float32).T.astype(np.float16))
    wx = np.ascontiguousarray(
        (np.asarray(inputs["conv1_w0_w"], np.float32)
         + np.asarray(inputs["conv1_w1_w"], np.float32)).T.astype(np.float16))
    bh = (np.asarray(inputs["conv1_wl_b"], np.float32)
          + np.asarray(inputs["conv1_w0_b"], np.float32)
          + np.asarray(inputs["conv1_w1_b"], np.float32)).reshape(H, 1)
    wo = np.ascontiguousarray(np.asarray(inputs["out_w"], np.float32).T.astype(np.float16))
    bo = np.asarray(inputs["out_b"], np.float32).reshape(1, H).astype(np.float16)
    boc = np.asarray(inputs["out_b"], np.float32).reshape(H, 1)
    iota = np.ascontiguousarray(
        np.tile(np.arange(W, dtype=np.float16)[None, :], (128, 1)))
    xa16 = np.ascontiguousarray(x_a.astype(np.float16))
    xa32 = xa16.view(np.float32)   # [N, 64] bit view

    budgets, group_order, CA, CB, per_core, rows_of = _pack_edges(dst, src)
    nc = _build_program(budgets, group_order, CA, CB)

    NCOL = NGROUP * GROUP
    in_maps = []
    for c in range(P):
        xn = np.zeros((H, NCOL), np.float16)
        xn[:, :NSH] = xa16[rows_of[c]].T
        # processing-order column layout for contiguous prefetch quanta
        xaT = np.empty((H, NCOL), np.float16)
        for gi, g in enumerate(group_order):
            xaT[:, gi * GROUP:(gi + 1) * GROUP] = xn[:, g * GROUP:(g + 1) * GROUP]
        a = per_core[c]
        in_maps.append({
            "xa32": xa32,
            "xaT": xaT,
            "idxA": _wrap_idx(a["idxA"]),
            "idxB": _wrap_idx(a["idxB"]),
            "destA": a["destA"],
            "destB": a["destB"],
            "wagg": wagg, "wx": wx, "wo": wo, "bh": bh, "bo": bo,
            "boc": boc,
            "ones": np.ones((1, GROUP), np.float16),
            "iota": iota,
        })
    return nc, in_maps, rows_of


def assemble(results, rows_of):
    out = np.empty((N, H), np.float32)
    for c in range(P):
        out[rows_of[c]] = results[c]["outT"].T.astype(np.float32)
    return out


def kernel(**inputs):
    from concourse.bass_utils import run_bass_kernel_spmd

    nc, in_maps, rows_of = prepare(inputs)
    r = run_bass_kernel_spmd(nc, in_maps, list(range(P)))
    return assemble(r.results, rows_of)
